# revision 23
# baseline (speedup 1.0000x reference)
"""Bass/Trainium2 kernel for single-token (decode) self-attention with a
large KV cache, RoPE, and output projection.

Sharding: tensor-parallel over heads. 16 heads / 8 cores = 2 heads per
core; every core sees all 8 batch rows. Per-core HBM traffic is dominated
by its KV-cache slice, so the cache is down-converted on the host (pure
input marshaling): both K and V to bf16 (fp8 V was tried and costs
~1.8e-2 rel err - quantization noise on V hits the context at full
strength, it does not average away). QKV weights are sliced by head
rows, Wo by columns (row-parallel out projection); each core returns a
partial (8, 1024) output and the host sums the 8 partials.

Kernel structure per core:
  - q/k/v = x @ W.T + b via PE in bf16; all four weight slices arrive as
    one host-packed tensor so a single DMA covers them. RoPE on DVE in
    fp32 (q rows also carry the 1/sqrt(hd) attention scale), then the
    per-batch payload [q | v0 v1 | exp(s_new)] is downcast to bf16 and
    broadcast to all 128 partitions via one-hot PE matmuls.
  - K slabs land with key j = 64*partition + j_col, one DMA per
    (batch, head) on the SP HWDGE ring; V is host-interleaved by head
    ([p, j, h, d], 8KB descriptor chunks = exactly two 4KB packets) and
    streams on the ACT HWDGE ring so the two rings' issue gaps overlap.
    K is prefetched one batch ahead.
  - scores: plain tensor_tensor multiply against a 0-stride broadcast
    view of q, then a 6-step in-place binary-tree reduction over hd, also
    tensor_tensor. TT is the only two-tensor DVE op that reaches the 2x
    packed-bf16 mode on TRN2 hardware (measured: scalar_tensor_tensor
    always runs 1x; tensor_reduce has no fast mode at all).
  - softmax without max subtraction (scores are O(1) by construction);
    exp on ACT to bf16 weights in a head-interleaved [j, h] layout, with
    accum_out collecting each head's per-partition weight sum.
  - attn @ V: 64 block-diagonal PE matmuls per batch (stationary =
    at[:, j, 0:2] covering BOTH heads, moving = [v_h0 | v_h1], 128 cols;
    the off-diagonal PSUM blocks are garbage that is never read). This
    halves PE instruction count - the PE sequencer sustains only
    ~55ns/instruction and was the v5 bottleneck. Denominators accumulate
    into two extra PSUM columns via tiny f32 matmuls over the exp accums.
  - normalize on DVE two batches behind the score pipeline (a selector
    matmul first merges head 1's row down to partition 0), PE-transpose
    the context row, out-projection partial via bf16 PE.
"""

import functools
import os
import sys

import numpy as np

for _p in ("/opt/trn_rl_repo", "/root/.axon_site/_ro/trn_rl_repo"):
    if os.path.isdir(_p) and _p not in sys.path:
        sys.path.insert(0, _p)

from contextlib import ExitStack

import ml_dtypes

import concourse.tile as tile
from concourse import bacc, mybir
from concourse.bass_utils import run_bass_kernel_spmd

B, S, D, H, PAST = 8, 1, 1024, 16, 8192
HD = 64
NCORES = 8
HPC = H // NCORES          # heads per core = 2
LP = HPC * HD              # local projection width = 128
NCOL = PAST // 128         # 64 keys per partition = score columns per pair
PW = 2 * LP + HPC         # 258: [q(128) | v0 v1 (128) | exp(s_new)(2)]

F32 = mybir.dt.float32
BF16 = mybir.dt.bfloat16
MULT = mybir.AluOpType.mult
ADD = mybir.AluOpType.add
EXP = mybir.ActivationFunctionType.Exp


def _build_bass():
    nc = bacc.Bacc(
        "TRN2", target_bir_lowering=False, debug=False, num_devices=NCORES
    )

    # packed weights: [wq | wk | wv | wo] along dim1, 8 chunks each
    d_ww = nc.dram_tensor("ww", (128, 32, 128), BF16, kind="ExternalInput").ap()
    d_xt = nc.dram_tensor("xt", (128, 8, B), BF16, kind="ExternalInput").ap()
    # c8: [rope(512) | bqkv(384)] fp32 ; eallb: one-hot bcast rows, bf16
    d_c8 = nc.dram_tensor("c8", (B, 896), F32, kind="ExternalInput").ap()
    d_eb = nc.dram_tensor("eb", (B, B * 128), BF16, kind="ExternalInput").ap()
    # c128: col 0 = ones (transpose identity), col 1 = e1 row selector
    d_c128 = nc.dram_tensor("c128", (128, 2), F32, kind="ExternalInput").ap()
    d_pk = nc.dram_tensor("pk", (B, HPC, PAST, HD), BF16, kind="ExternalInput").ap()
    # V interleaved by head: [b, partition, j, h, d] so one matmul can
    # stream both heads' V rows for a key column (64-wide rows keep DMA
    # descriptors at exactly two 4KB packets)
    d_pv = nc.dram_tensor("pv", (B, 128, NCOL, HPC, HD), BF16, kind="ExternalInput").ap()
    d_out = nc.dram_tensor("out", (B, D), F32, kind="ExternalOutput").ap()

    with tile.TileContext(nc) as tc:
        with ExitStack() as ctx:
            const = ctx.enter_context(tc.tile_pool(name="const", bufs=1))
            small = ctx.enter_context(tc.tile_pool(name="small", bufs=1))
            tiny = ctx.enter_context(tc.tile_pool(name="tiny", bufs=2))
            wt = ctx.enter_context(tc.tile_pool(name="wt", bufs=1))
            kpool = ctx.enter_context(tc.tile_pool(name="kpool", bufs=4))
            vpool = ctx.enter_context(tc.tile_pool(name="vpool", bufs=4))
            prpool = ctx.enter_context(tc.tile_pool(name="prpool", bufs=2))
            atpool = ctx.enter_context(tc.tile_pool(name="atpool", bufs=2))

            # ---- constants + weights (ordered so the prologue chain can
            # start as early as possible) ----------------------------------
            wall = wt.tile([128, 32, 128], BF16)
            nc.sync.dma_start(wall[:], d_ww[:])
            xt = small.tile([128, 8, B], BF16)
            nc.scalar.dma_start(xt[:], d_xt[:])
            c8 = const.tile([B, 896], F32)
            nc.scalar.dma_start(c8[:], d_c8[:])
            eallb = const.tile([B, B * 128], BF16)
            nc.scalar.dma_start(eallb[:], d_eb[:])
            c128 = const.tile([128, 2], F32)
            nc.scalar.dma_start(c128[:], d_c128[:])
            ident = c128[:, 0:1]
            e1sel = c128[0:2, 1:2]
            rope = c8[:, 0:512]
            bias = c8[:, 512:896]
            wot = wall[:, 24:32, :]

            # prefetch K/V for batch 0 ahead of the prologue compute
            kts = {}
            vts = {}

            def fetch(b):
                kt = kpool.tile([128, HPC, NCOL, HD], BF16, tag="kt")
                ksrc = d_pk[b].rearrange("h (p j) d -> p h j d", p=128)
                nc.sync.dma_start(kt[:, 0], ksrc[:, 0])
                nc.sync.dma_start(kt[:, 1], ksrc[:, 1])
                kts[b] = kt
                vt = vpool.tile([128, NCOL, HPC, HD], BF16, tag="vt")
                nc.scalar.dma_start(
                    vt[:, 0 : NCOL // 2], d_pv[b, :, 0 : NCOL // 2]
                )
                nc.scalar.dma_start(
                    vt[:, NCOL // 2 :], d_pv[b, :, NCOL // 2 :]
                )
                vts[b] = vt

            fetch(0)

            # ---- prologue: projections, RoPE, payload broadcast -----------
            qb = const.tile([128, B * PW], BF16)
            with ExitStack() as pctx:
                ps_p = pctx.enter_context(
                    tc.tile_pool(name="ps_p", bufs=1, space="PSUM")
                )
                ps_bc = pctx.enter_context(
                    tc.tile_pool(name="ps_bc", bufs=2, space="PSUM")
                )

                # qkv projection: (8, 384) = x @ [Wq|Wk|Wv].T
                qkv_ps = ps_p.tile([B, 3 * LP], F32, tag="qkv_ps")
                for i in range(3):
                    for j in range(8):
                        nc.tensor.matmul(
                            qkv_ps[:, LP * i : LP * (i + 1)],
                            xt[:, j, :],
                            wall[:, 8 * i + j, :],
                            start=(j == 0),
                            stop=(j == 7),
                        )
                qkv = small.tile([B, 3 * LP], F32)
                nc.vector.tensor_tensor(qkv[:], qkv_ps[:], bias[:], ADD)

                # RoPE on q and k (fp32): rot = qk * C + swapped(qk) * S
                rot = small.tile([B, 2 * LP], F32)
                swp = small.tile([B, 2 * LP], F32)
                for i in range(2):  # q, k
                    src = qkv[:, LP * i : LP * (i + 1)].rearrange(
                        "p (h t f) -> p h t f", h=HPC, t=2
                    )
                    dst = swp[:, LP * i : LP * (i + 1)].rearrange(
                        "p (h t f) -> p h t f", h=HPC, t=2
                    )
                    nc.vector.tensor_copy(dst[:, :, 0, :], src[:, :, 1, :])
                    nc.vector.tensor_copy(dst[:, :, 1, :], src[:, :, 0, :])
                tmp = small.tile([B, 2 * LP], F32)
                nc.vector.tensor_tensor(tmp[:], swp[:], rope[:, 256:512], MULT)
                nc.vector.tensor_tensor(
                    rot[:], qkv[:, 0 : 2 * LP], rope[:, 0:256], MULT
                )
                nc.vector.tensor_tensor(rot[:], rot[:], tmp[:], ADD)

                # new-token scores s_new = rot(q) . rot(k) per head
                # (q side is pre-scaled by 0.125 via the rope tables)
                snew = small.tile([B, HPC], F32)
                sttp = small.tile([B, HD], F32)
                for hp in range(HPC):
                    nc.vector.scalar_tensor_tensor(
                        out=sttp[:],
                        in0=rot[:, LP + HD * hp : LP + HD * (hp + 1)],
                        scalar=1.0,
                        in1=rot[:, HD * hp : HD * (hp + 1)],
                        op0=MULT,
                        op1=MULT,
                        accum_out=snew[:, hp : hp + 1],
                    )

                # payload (bf16): [q(128) | v0 1 | v1 1 | exp(s_new)(2)]
                payb = small.tile([B, PW], BF16)
                nc.vector.tensor_copy(payb[:, 0:LP], rot[:, 0:LP])
                nc.vector.tensor_copy(
                    payb[:, LP : 2 * LP], qkv[:, 2 * LP : 3 * LP]
                )
                nc.scalar.activation(
                    payb[:, PW - HPC : PW], snew[:], EXP
                )

                # broadcast payload rows to all 128 partitions
                for b in range(B):
                    bc = ps_bc.tile([128, PW], F32, tag="bc")
                    nc.tensor.matmul(
                        bc[:],
                        eallb[:, 128 * b : 128 * (b + 1)],
                        payb[:],
                        start=True,
                        stop=True,
                    )
                    nc.scalar.copy(qb[:, PW * b : PW * (b + 1)], bc[:])

            # ---- main loop: one iteration per batch row (2 heads each) ----
            ps_ctx = ctx.enter_context(
                tc.tile_pool(name="ps_ctx", bufs=3, space="PSUM")
            )
            ps_m = ctx.enter_context(
                tc.tile_pool(name="ps_m", bufs=2, space="PSUM")
            )
            ps_t = ctx.enter_context(
                tc.tile_pool(name="ps_t", bufs=1, space="PSUM")
            )
            ps_o = ctx.enter_context(
                tc.tile_pool(name="ps_o", bufs=2, space="PSUM")
            )
            ctxT_ps = ps_t.tile([128, B], F32)
            ctxn = small.tile([1, B * LP], F32)
            ctxbs = {}

            def epilogue(b):
                # ctxb is [2, 128]: head 0 context on row 0 cols 0:64, head 1
                # on row 1 cols 64:128 (block-diagonal accumulation; the
                # off-diagonal blocks are never read). Merge row 1 down to
                # partition 0 via a selector matmul, then normalize by the
                # denominators collected in den.
                ctxb = ctxbs.pop(b)
                sb = tiny.tile([HPC, HPC * HD + HPC], F32, tag="sb")
                nc.scalar.copy(sb[:], ctxb[:])
                m1 = ps_m.tile([1, HPC * HD + HPC], F32, tag="m1")
                nc.tensor.matmul(m1[:], e1sel, sb[:], start=True, stop=True)
                rec = tiny.tile([1, HPC], F32, tag="rec")
                nc.vector.reciprocal(rec[:], ctxb[0:1, HPC * HD : HPC * HD + HPC])
                nc.vector.tensor_scalar_mul(
                    ctxn[0:1, LP * b : LP * b + HD],
                    sb[0:1, 0:HD],
                    rec[0:1, 0:1],
                )
                nc.vector.tensor_scalar_mul(
                    ctxn[0:1, LP * b + HD : LP * (b + 1)],
                    m1[0:1, HD : 2 * HD],
                    rec[0:1, 1:2],
                )
                nc.tensor.transpose(
                    ctxT_ps[:, b : b + 1],
                    ctxn[0:1, LP * b : LP * (b + 1)],
                    ident[0:1, 0:1],
                )

            fetch(1)
            for b in range(B):
                q0 = PW * b
                kt = kts.pop(b)
                vt = vts.pop(b)
                if b + 2 < B:
                    fetch(b + 2)

                # scores: prod = kt * q (2x DVE) per head, then one fused
                # in-place tree-reduce over d covering both heads
                prod = prpool.tile([128, HPC, NCOL, HD], BF16, tag="prod")
                at = atpool.tile([128, NCOL, HPC], BF16, tag="at")
                acc = atpool.tile([128, HPC], F32, tag="acc")
                for h in range(HPC):
                    qv = qb[:, q0 + HD * h : q0 + HD * (h + 1)].rearrange(
                        "p (o d) -> p o d", o=1
                    ).broadcast_to([128, NCOL, HD])
                    nc.vector.tensor_tensor(prod[:, h], kt[:, h], qv, MULT)
                # normalize batch b-2 now: early in the queues so its
                # ACT copy is not stuck behind this iteration's exp and its
                # DVE ops slot in right after the tree
                if b >= 2:
                    epilogue(b - 2)
                pf = prod.rearrange("p h j d -> p (h j) d")
                w = HD // 2
                while w >= 1:
                    nc.vector.tensor_tensor(
                        pf[:, :, 0:w], pf[:, :, 0:w], pf[:, :, w : 2 * w], ADD
                    )
                    w //= 2
                # exp into the head-interleaved at layout; accum_out gives
                # each head's per-partition weight sum for the denominator
                for h in range(HPC):
                    nc.scalar.activation(
                        at[:, :, h],
                        prod[:, h, :, 0],
                        EXP,
                        accum_out=acc[:, h : h + 1],
                    )

                # attn @ V on PE, both heads per matmul (block-diagonal):
                # stationary at[:, j, 0:2], moving [v_h0 | v_h1] (128 cols)
                # cols 0:128 = block-diagonal context, 128:130 = denoms
                ctxb = ps_ctx.tile([HPC, HPC * HD + HPC], F32, tag="ctxb")
                ctxbs[b] = ctxb
                for j in range(NCOL):
                    nc.tensor.matmul(
                        ctxb[:, 0 : HPC * HD],
                        at[:, j, :],
                        vt[:, j],
                        start=(j == 0),
                        stop=False,
                    )
                # new token, both heads at once: lhsT = [es0 es1],
                # rhs = [v0 | v1]
                nc.tensor.matmul(
                    ctxb[:, 0 : HPC * HD],
                    qb[0:1, q0 + PW - HPC : q0 + PW],
                    qb[0:1, q0 + LP : q0 + 2 * LP],
                    start=False,
                    stop=True,
                )
                # denominators: cross-partition sum of acc plus exp(s_new)
                for h in range(HPC):
                    dsl = ctxb[0:1, HPC * HD + h : HPC * HD + h + 1]
                    nc.tensor.matmul(
                        dsl, acc[:, h : h + 1], ident, start=True, stop=False
                    )
                    nc.tensor.matmul(
                        dsl,
                        eallb[0:1, 0:1],
                        qb[0:1, q0 + PW - HPC + h : q0 + PW - HPC + h + 1],
                        start=False,
                        stop=True,
                    )

            epilogue(B - 2)
            epilogue(B - 1)

            # ---- finalize: transpose is done; out-projection ---------------
            ctxT = small.tile([128, B], BF16)
            nc.scalar.copy(ctxT[:], ctxT_ps[:])

            outsb = small.tile([B, D], F32)
            for half in range(2):
                op_ps = ps_o.tile([B, 512], F32, tag="op_ps")
                nc.tensor.matmul(
                    op_ps[:],
                    ctxT[:],
                    wot[:, 4 * half : 4 * (half + 1), :],
                    start=True,
                    stop=True,
                )
                nc.vector.tensor_copy(
                    outsb[:, 512 * half : 512 * (half + 1)], op_ps[:]
                )
                nc.sync.dma_start(
                    d_out[:, 512 * half : 512 * (half + 1)],
                    outsb[:, 512 * half : 512 * (half + 1)],
                )

    nc.compile()
    return nc


@functools.lru_cache(maxsize=1)
def _get_nc():
    return _build_bass()


def _rope_tables():
    """cos/sin rows for position PAST, mirroring reference.py's fp32 jax
    arithmetic so the tables round identically."""
    import jax
    import jax.numpy as jnp

    pos = (PAST + jnp.arange(S)).astype(jnp.float32)
    inv_freq = 1.0 / (
        10000.0 ** (jnp.arange(0, HD, 2, dtype=jnp.float32) / HD)
    )
    ang = pos[:, None] * inv_freq[None, :]
    cos32 = np.asarray(jnp.cos(ang))[0]
    sin32 = np.asarray(jnp.sin(ang))[0]
    cos64 = np.concatenate([cos32, cos32])
    ssin64 = np.concatenate([-sin32, sin32])
    return cos64.astype(np.float32), ssin64.astype(np.float32)


def _install_ntff_hook_shim():
    """The agent image's antenv stub lacks axon_hooks, which degrades
    run_bass_kernel_spmd(trace=True) into an ImportError. Provide the
    module and register the ctypes-based NTFF hook from trn_agent_boot."""
    import types

    try:
        import antenv.axon_hooks  # noqa: F401

        return
    except ImportError:
        pass
    try:
        import antenv
        from trn_agent_boot.trn_boot import _ntff_profile_via_ctypes

        mod = types.ModuleType("antenv.axon_hooks")
        _state = {"hook": _ntff_profile_via_ctypes("/opt/axon/libaxon_pjrt.so")}
        mod.get_axon_ntff_profile_hook = lambda: _state["hook"]
        mod.set_axon_ntff_profile_hook = lambda h: _state.update(hook=h)
        sys.modules["antenv.axon_hooks"] = mod
        antenv.axon_hooks = mod
    except Exception as e:  # profiling is best-effort
        print(f"ntff hook shim failed: {e}", file=sys.stderr)


def kernel(x, Wq, bq, Wk, bk, Wv, bv, Wo, bo, past_k, past_v):
    x = np.asarray(x, np.float32).reshape(B, D)
    Wq = np.asarray(Wq, np.float32)
    Wk = np.asarray(Wk, np.float32)
    Wv = np.asarray(Wv, np.float32)
    Wo = np.asarray(Wo, np.float32)
    bq = np.asarray(bq, np.float32)
    bk = np.asarray(bk, np.float32)
    bv = np.asarray(bv, np.float32)
    bo = np.asarray(bo, np.float32)
    past_k = np.asarray(past_k, np.float32)
    past_v = np.asarray(past_v, np.float32)

    bf16 = ml_dtypes.bfloat16

    cos64, ssin64 = _rope_tables()
    # C/S for the q columns carry the 1/sqrt(hd) attention scale
    cq = np.tile(cos64, HPC) * np.float32(0.125)
    ck = np.tile(cos64, HPC)
    sq = np.tile(ssin64, HPC) * np.float32(0.125)
    sk = np.tile(ssin64, HPC)
    rope = np.tile(
        np.concatenate([cq, ck, sq, sk])[None, :], (B, 1)
    ).astype(np.float32)
    eall = np.zeros((B, B * 128), np.float32)
    for b in range(B):
        eall[b, 128 * b : 128 * (b + 1)] = 1.0
    c128 = np.ones((128, 2), np.float32)
    c128[:, 1] = 0.0
    c128[1, 1] = 1.0

    # weight layout: [partition=in-chunk-row, j=in-chunk, out-col],
    # contiguous per partition so DMA descriptors are large
    def wlay(w_rows):  # w_rows: (128, 1024) slice of W (rows = this core)
        return w_rows.T.reshape(8, 128, 128).transpose(1, 0, 2)

    xtl = np.ascontiguousarray(
        x.T.reshape(8, 128, B).transpose(1, 0, 2)
    ).astype(bf16)

    in_maps = []
    for c in range(NCORES):
        hs = slice(HPC * c, HPC * (c + 1))
        rs = slice(LP * c, LP * (c + 1))
        bqkv = np.tile(
            np.concatenate([bq[rs], bk[rs], bv[rs]])[None, :], (B, 1)
        ).astype(np.float32)
        c8 = np.concatenate([rope, bqkv], axis=1).astype(np.float32)
        ww = np.concatenate(
            [
                wlay(Wq[rs]),
                wlay(Wk[rs]),
                wlay(Wv[rs]),
                Wo[:, rs].reshape(8, 128, LP).transpose(2, 0, 1),
            ],
            axis=1,
        ).astype(bf16)
        # head-interleaved V: [b, partition, j, h, d]
        pvt = np.ascontiguousarray(
            past_v[:, hs]
            .reshape(B, HPC, 128, NCOL, HD)
            .transpose(0, 2, 3, 1, 4)
        ).astype(bf16)
        in_maps.append(
            {
                "xt": xtl,
                "ww": np.ascontiguousarray(ww),
                "c8": c8,
                "eb": eall.astype(bf16),
                "c128": c128,
                "pk": np.ascontiguousarray(past_k[:, hs]).astype(bf16),
                "pv": pvt,
            }
        )

    nc = _get_nc()
    trace = bool(int(os.environ.get("KERNEL_TRACE", "0")))
    if trace:
        _install_ntff_hook_shim()
    res = run_bass_kernel_spmd(
        nc, in_maps, core_ids=list(range(NCORES)), trace=trace
    )
    kernel.last_results = res

    partial = np.zeros((B, D), np.float32)
    for c in range(NCORES):
        partial = partial + res.results[c]["out"]
    out = partial + bo[None, :]
    return out.reshape(B, S, D).astype(np.float32)


# revision 25
# speedup vs baseline: 1.0229x; 1.0229x over previous
"""Bass/Trainium2 kernel for single-token (decode) self-attention with a
large KV cache, RoPE, and output projection.

Sharding: tensor-parallel over heads. 16 heads / 8 cores = 2 heads per
core; every core sees all 8 batch rows. Per-core HBM traffic is dominated
by its KV-cache slice, so the cache is down-converted on the host (pure
input marshaling): both K and V to bf16 (fp8 V was tried and costs
~1.8e-2 rel err - quantization noise on V hits the context at full
strength, it does not average away). QKV weights are sliced by head
rows, Wo by columns (row-parallel out projection); each core returns a
partial (8, 1024) output and the host sums the 8 partials.

Kernel structure per core:
  - q/k/v = x @ W.T + b via PE in bf16; all four weight slices arrive as
    one host-packed tensor so a single DMA covers them. RoPE on DVE in
    fp32 (q rows also carry the 1/sqrt(hd) attention scale), then the
    per-batch payload [q | v0 v1 | exp(s_new)] is downcast to bf16 and
    broadcast to all 128 partitions via one-hot PE matmuls.
  - K slabs land with key j = 64*partition + j_col, one DMA per
    (batch, head) on the SP HWDGE ring; V is host-interleaved by head
    ([p, j, h, d], 8KB descriptor chunks = exactly two 4KB packets) and
    streams on the ACT HWDGE ring so the two rings' issue gaps overlap.
    K is prefetched one batch ahead.
  - scores: plain tensor_tensor multiply against a 0-stride broadcast
    view of q, then a 6-step in-place binary-tree reduction over hd, also
    tensor_tensor. TT is the only two-tensor DVE op that reaches the 2x
    packed-bf16 mode on TRN2 hardware (measured: scalar_tensor_tensor
    always runs 1x; tensor_reduce has no fast mode at all).
  - softmax without max subtraction (scores are O(1) by construction);
    exp on ACT to bf16 weights in a head-interleaved [j, h] layout, with
    accum_out collecting each head's per-partition weight sum.
  - attn @ V: 64 block-diagonal PE matmuls per batch (stationary =
    at[:, j, 0:2] covering BOTH heads, moving = [v_h0 | v_h1], 128 cols;
    the off-diagonal PSUM blocks are garbage that is never read). This
    halves PE instruction count - the PE sequencer sustains only
    ~55ns/instruction and was the v5 bottleneck. Denominators accumulate
    into two extra PSUM columns via tiny f32 matmuls over the exp accums.
  - normalize on DVE two batches behind the score pipeline (a selector
    matmul first merges head 1's row down to partition 0), PE-transpose
    the context row, out-projection partial via bf16 PE.
"""

import functools
import os
import sys

import numpy as np

for _p in ("/opt/trn_rl_repo", "/root/.axon_site/_ro/trn_rl_repo"):
    if os.path.isdir(_p) and _p not in sys.path:
        sys.path.insert(0, _p)

from contextlib import ExitStack

import ml_dtypes

import concourse.tile as tile
from concourse import bacc, mybir
from concourse.bass_utils import run_bass_kernel_spmd

B, S, D, H, PAST = 8, 1, 1024, 16, 8192
HD = 64
NCORES = 8
HPC = H // NCORES          # heads per core = 2
LP = HPC * HD              # local projection width = 128
NCOL = PAST // 128         # 64 keys per partition = score columns per pair
PW = 2 * LP + HPC         # 258: [q(128) | v0 v1 (128) | exp(s_new)(2)]

F32 = mybir.dt.float32
BF16 = mybir.dt.bfloat16
MULT = mybir.AluOpType.mult
ADD = mybir.AluOpType.add
EXP = mybir.ActivationFunctionType.Exp


def _build_bass():
    nc = bacc.Bacc(
        "TRN2", target_bir_lowering=False, debug=False, num_devices=NCORES
    )

    # packed weights: [wq | wk | wv | wo] along dim1, 8 chunks each
    d_ww = nc.dram_tensor("ww", (128, 32, 128), BF16, kind="ExternalInput").ap()
    d_xt = nc.dram_tensor("xt", (128, 8, B), BF16, kind="ExternalInput").ap()
    # c8: [rope(512) | bqkv(384)] fp32 ; eallb: one-hot bcast rows, bf16
    d_c8 = nc.dram_tensor("c8", (B, 896), F32, kind="ExternalInput").ap()
    d_eb = nc.dram_tensor("eb", (B, B * 128), BF16, kind="ExternalInput").ap()
    # c128: col 0 = ones (transpose identity), col 1 = e1 row selector
    d_c128 = nc.dram_tensor("c128", (128, 2), F32, kind="ExternalInput").ap()
    d_pk = nc.dram_tensor("pk", (B, HPC, PAST, HD), BF16, kind="ExternalInput").ap()
    # V interleaved by head: [b, partition, j, h, d] so one matmul can
    # stream both heads' V rows for a key column (64-wide rows keep DMA
    # descriptors at exactly two 4KB packets)
    d_pv = nc.dram_tensor("pv", (B, 128, NCOL, HPC, HD), BF16, kind="ExternalInput").ap()
    d_out = nc.dram_tensor("out", (B, D), F32, kind="ExternalOutput").ap()

    with tile.TileContext(nc) as tc:
        with ExitStack() as ctx:
            const = ctx.enter_context(tc.tile_pool(name="const", bufs=1))
            small = ctx.enter_context(tc.tile_pool(name="small", bufs=1))
            tiny = ctx.enter_context(tc.tile_pool(name="tiny", bufs=2))
            wt = ctx.enter_context(tc.tile_pool(name="wt", bufs=1))
            kpool = ctx.enter_context(tc.tile_pool(name="kpool", bufs=5))
            vpool = ctx.enter_context(tc.tile_pool(name="vpool", bufs=3))
            prpool = ctx.enter_context(tc.tile_pool(name="prpool", bufs=2))
            atpool = ctx.enter_context(tc.tile_pool(name="atpool", bufs=2))

            # ---- constants + weights (ordered so the prologue chain can
            # start as early as possible) ----------------------------------
            wall = wt.tile([128, 32, 128], BF16)
            nc.sync.dma_start(wall[:], d_ww[:])
            xt = small.tile([128, 8, B], BF16)
            nc.scalar.dma_start(xt[:], d_xt[:])
            c8 = const.tile([B, 896], F32)
            nc.scalar.dma_start(c8[:], d_c8[:])
            eallb = const.tile([B, B * 128], BF16)
            nc.scalar.dma_start(eallb[:], d_eb[:])
            c128 = const.tile([128, 2], F32)
            nc.scalar.dma_start(c128[:], d_c128[:])
            ident = c128[:, 0:1]
            e1sel = c128[0:2, 1:2]
            rope = c8[:, 0:512]
            bias = c8[:, 512:896]
            wot = wall[:, 24:32, :]

            # prefetch K/V for batch 0 ahead of the prologue compute
            kts = {}
            vts = {}

            def fetch(b):
                # one fat DMA instruction per tensor per batch: fewer ring
                # issue gaps; descriptors stay 8KB (K, per head) / 16KB (V)
                kt = kpool.tile([128, HPC, NCOL, HD], BF16, tag="kt")
                ksrc = d_pk[b].rearrange("h (p j) d -> p h j d", p=128)
                nc.sync.dma_start(kt[:], ksrc[:])
                kts[b] = kt
                vt = vpool.tile([128, NCOL, HPC, HD], BF16, tag="vt")
                nc.scalar.dma_start(vt[:], d_pv[b])
                vts[b] = vt

            fetch(0)

            # ---- prologue: projections, RoPE, payload broadcast -----------
            qb = const.tile([128, B * PW], BF16)
            with ExitStack() as pctx:
                ps_p = pctx.enter_context(
                    tc.tile_pool(name="ps_p", bufs=1, space="PSUM")
                )
                ps_bc = pctx.enter_context(
                    tc.tile_pool(name="ps_bc", bufs=2, space="PSUM")
                )

                # qkv projection: (8, 384) = x @ [Wq|Wk|Wv].T
                qkv_ps = ps_p.tile([B, 3 * LP], F32, tag="qkv_ps")
                for i in range(3):
                    for j in range(8):
                        nc.tensor.matmul(
                            qkv_ps[:, LP * i : LP * (i + 1)],
                            xt[:, j, :],
                            wall[:, 8 * i + j, :],
                            start=(j == 0),
                            stop=(j == 7),
                        )
                qkv = small.tile([B, 3 * LP], F32)
                nc.vector.tensor_tensor(qkv[:], qkv_ps[:], bias[:], ADD)

                # RoPE on q and k (fp32): rot = qk * C + swapped(qk) * S
                rot = small.tile([B, 2 * LP], F32)
                swp = small.tile([B, 2 * LP], F32)
                for i in range(2):  # q, k
                    src = qkv[:, LP * i : LP * (i + 1)].rearrange(
                        "p (h t f) -> p h t f", h=HPC, t=2
                    )
                    dst = swp[:, LP * i : LP * (i + 1)].rearrange(
                        "p (h t f) -> p h t f", h=HPC, t=2
                    )
                    nc.vector.tensor_copy(dst[:, :, 0, :], src[:, :, 1, :])
                    nc.vector.tensor_copy(dst[:, :, 1, :], src[:, :, 0, :])
                tmp = small.tile([B, 2 * LP], F32)
                nc.vector.tensor_tensor(tmp[:], swp[:], rope[:, 256:512], MULT)
                nc.vector.tensor_tensor(
                    rot[:], qkv[:, 0 : 2 * LP], rope[:, 0:256], MULT
                )
                nc.vector.tensor_tensor(rot[:], rot[:], tmp[:], ADD)

                # new-token scores s_new = rot(q) . rot(k) per head
                # (q side is pre-scaled by 0.125 via the rope tables)
                snew = small.tile([B, HPC], F32)
                sttp = small.tile([B, HD], F32)
                for hp in range(HPC):
                    nc.vector.scalar_tensor_tensor(
                        out=sttp[:],
                        in0=rot[:, LP + HD * hp : LP + HD * (hp + 1)],
                        scalar=1.0,
                        in1=rot[:, HD * hp : HD * (hp + 1)],
                        op0=MULT,
                        op1=MULT,
                        accum_out=snew[:, hp : hp + 1],
                    )

                # payload (bf16): [q(128) | v0 1 | v1 1 | exp(s_new)(2)]
                payb = small.tile([B, PW], BF16)
                nc.vector.tensor_copy(payb[:, 0:LP], rot[:, 0:LP])
                nc.vector.tensor_copy(
                    payb[:, LP : 2 * LP], qkv[:, 2 * LP : 3 * LP]
                )
                nc.scalar.activation(
                    payb[:, PW - HPC : PW], snew[:], EXP
                )

                # broadcast payload rows to all 128 partitions
                for b in range(B):
                    bc = ps_bc.tile([128, PW], F32, tag="bc")
                    nc.tensor.matmul(
                        bc[:],
                        eallb[:, 128 * b : 128 * (b + 1)],
                        payb[:],
                        start=True,
                        stop=True,
                    )
                    nc.scalar.copy(qb[:, PW * b : PW * (b + 1)], bc[:])

            # ---- main loop: one iteration per batch row (2 heads each) ----
            ps_ctx = ctx.enter_context(
                tc.tile_pool(name="ps_ctx", bufs=3, space="PSUM")
            )
            ps_m = ctx.enter_context(
                tc.tile_pool(name="ps_m", bufs=2, space="PSUM")
            )
            ps_t = ctx.enter_context(
                tc.tile_pool(name="ps_t", bufs=1, space="PSUM")
            )
            ps_o = ctx.enter_context(
                tc.tile_pool(name="ps_o", bufs=2, space="PSUM")
            )
            ctxT_ps = ps_t.tile([128, B], F32)
            ctxn = small.tile([1, B * LP], F32)
            ctxbs = {}

            def epilogue(b):
                # ctxb is [2, 128]: head 0 context on row 0 cols 0:64, head 1
                # on row 1 cols 64:128 (block-diagonal accumulation; the
                # off-diagonal blocks are never read). Merge row 1 down to
                # partition 0 via a selector matmul, then normalize by the
                # denominators collected in den.
                ctxb = ctxbs.pop(b)
                sb = tiny.tile([HPC, HPC * HD + HPC], F32, tag="sb")
                nc.scalar.copy(sb[:], ctxb[:])
                m1 = ps_m.tile([1, HPC * HD + HPC], F32, tag="m1")
                nc.tensor.matmul(m1[:], e1sel, sb[:], start=True, stop=True)
                rec = tiny.tile([1, HPC], F32, tag="rec")
                nc.vector.reciprocal(rec[:], ctxb[0:1, HPC * HD : HPC * HD + HPC])
                nc.vector.tensor_scalar_mul(
                    ctxn[0:1, LP * b : LP * b + HD],
                    sb[0:1, 0:HD],
                    rec[0:1, 0:1],
                )
                nc.vector.tensor_scalar_mul(
                    ctxn[0:1, LP * b + HD : LP * (b + 1)],
                    m1[0:1, HD : 2 * HD],
                    rec[0:1, 1:2],
                )
                nc.tensor.transpose(
                    ctxT_ps[:, b : b + 1],
                    ctxn[0:1, LP * b : LP * (b + 1)],
                    ident[0:1, 0:1],
                )

            for b in range(B):
                q0 = PW * b
                kt = kts.pop(b)
                vt = vts.pop(b)
                if b + 1 < B:
                    fetch(b + 1)

                # scores: prod = kt * q (2x DVE) per head, then one fused
                # in-place tree-reduce over d covering both heads
                prod = prpool.tile([128, HPC, NCOL, HD], BF16, tag="prod")
                at = atpool.tile([128, NCOL, HPC], BF16, tag="at")
                acc = atpool.tile([128, HPC], F32, tag="acc")
                for h in range(HPC):
                    qv = qb[:, q0 + HD * h : q0 + HD * (h + 1)].rearrange(
                        "p (o d) -> p o d", o=1
                    ).broadcast_to([128, NCOL, HD])
                    nc.vector.tensor_tensor(prod[:, h], kt[:, h], qv, MULT)
                # normalize batch b-2 now: early in the queues so its
                # ACT copy is not stuck behind this iteration's exp and its
                # DVE ops slot in right after the tree
                if b >= 2:
                    epilogue(b - 2)
                pf = prod.rearrange("p h j d -> p (h j) d")
                w = HD // 2
                while w >= 1:
                    nc.vector.tensor_tensor(
                        pf[:, :, 0:w], pf[:, :, 0:w], pf[:, :, w : 2 * w], ADD
                    )
                    w //= 2
                # exp into the head-interleaved at layout; accum_out gives
                # each head's per-partition weight sum for the denominator
                for h in range(HPC):
                    nc.scalar.activation(
                        at[:, :, h],
                        prod[:, h, :, 0],
                        EXP,
                        accum_out=acc[:, h : h + 1],
                    )

                # attn @ V on PE, both heads per matmul (block-diagonal):
                # stationary at[:, j, 0:2], moving [v_h0 | v_h1] (128 cols)
                # cols 0:128 = block-diagonal context, 128:130 = denoms
                ctxb = ps_ctx.tile([HPC, HPC * HD + HPC], F32, tag="ctxb")
                ctxbs[b] = ctxb
                for j in range(NCOL):
                    nc.tensor.matmul(
                        ctxb[:, 0 : HPC * HD],
                        at[:, j, :],
                        vt[:, j],
                        start=(j == 0),
                        stop=False,
                    )
                # new token, both heads at once: lhsT = [es0 es1],
                # rhs = [v0 | v1]
                nc.tensor.matmul(
                    ctxb[:, 0 : HPC * HD],
                    qb[0:1, q0 + PW - HPC : q0 + PW],
                    qb[0:1, q0 + LP : q0 + 2 * LP],
                    start=False,
                    stop=True,
                )
                # denominators: cross-partition sum of acc plus exp(s_new)
                for h in range(HPC):
                    dsl = ctxb[0:1, HPC * HD + h : HPC * HD + h + 1]
                    nc.tensor.matmul(
                        dsl, acc[:, h : h + 1], ident, start=True, stop=False
                    )
                    nc.tensor.matmul(
                        dsl,
                        eallb[0:1, 0:1],
                        qb[0:1, q0 + PW - HPC + h : q0 + PW - HPC + h + 1],
                        start=False,
                        stop=True,
                    )

            epilogue(B - 2)
            epilogue(B - 1)

            # ---- finalize: transpose is done; out-projection ---------------
            ctxT = small.tile([128, B], BF16)
            nc.scalar.copy(ctxT[:], ctxT_ps[:])

            outsb = small.tile([B, D], F32)
            for half in range(2):
                op_ps = ps_o.tile([B, 512], F32, tag="op_ps")
                nc.tensor.matmul(
                    op_ps[:],
                    ctxT[:],
                    wot[:, 4 * half : 4 * (half + 1), :],
                    start=True,
                    stop=True,
                )
                nc.vector.tensor_copy(
                    outsb[:, 512 * half : 512 * (half + 1)], op_ps[:]
                )
                nc.sync.dma_start(
                    d_out[:, 512 * half : 512 * (half + 1)],
                    outsb[:, 512 * half : 512 * (half + 1)],
                )

    nc.compile()
    return nc


@functools.lru_cache(maxsize=1)
def _get_nc():
    return _build_bass()


def _rope_tables():
    """cos/sin rows for position PAST, mirroring reference.py's fp32 jax
    arithmetic so the tables round identically."""
    import jax
    import jax.numpy as jnp

    pos = (PAST + jnp.arange(S)).astype(jnp.float32)
    inv_freq = 1.0 / (
        10000.0 ** (jnp.arange(0, HD, 2, dtype=jnp.float32) / HD)
    )
    ang = pos[:, None] * inv_freq[None, :]
    cos32 = np.asarray(jnp.cos(ang))[0]
    sin32 = np.asarray(jnp.sin(ang))[0]
    cos64 = np.concatenate([cos32, cos32])
    ssin64 = np.concatenate([-sin32, sin32])
    return cos64.astype(np.float32), ssin64.astype(np.float32)


def _install_ntff_hook_shim():
    """The agent image's antenv stub lacks axon_hooks, which degrades
    run_bass_kernel_spmd(trace=True) into an ImportError. Provide the
    module and register the ctypes-based NTFF hook from trn_agent_boot."""
    import types

    try:
        import antenv.axon_hooks  # noqa: F401

        return
    except ImportError:
        pass
    try:
        import antenv
        from trn_agent_boot.trn_boot import _ntff_profile_via_ctypes

        mod = types.ModuleType("antenv.axon_hooks")
        _state = {"hook": _ntff_profile_via_ctypes("/opt/axon/libaxon_pjrt.so")}
        mod.get_axon_ntff_profile_hook = lambda: _state["hook"]
        mod.set_axon_ntff_profile_hook = lambda h: _state.update(hook=h)
        sys.modules["antenv.axon_hooks"] = mod
        antenv.axon_hooks = mod
    except Exception as e:  # profiling is best-effort
        print(f"ntff hook shim failed: {e}", file=sys.stderr)


def kernel(x, Wq, bq, Wk, bk, Wv, bv, Wo, bo, past_k, past_v):
    x = np.asarray(x, np.float32).reshape(B, D)
    Wq = np.asarray(Wq, np.float32)
    Wk = np.asarray(Wk, np.float32)
    Wv = np.asarray(Wv, np.float32)
    Wo = np.asarray(Wo, np.float32)
    bq = np.asarray(bq, np.float32)
    bk = np.asarray(bk, np.float32)
    bv = np.asarray(bv, np.float32)
    bo = np.asarray(bo, np.float32)
    past_k = np.asarray(past_k, np.float32)
    past_v = np.asarray(past_v, np.float32)

    bf16 = ml_dtypes.bfloat16

    cos64, ssin64 = _rope_tables()
    # C/S for the q columns carry the 1/sqrt(hd) attention scale
    cq = np.tile(cos64, HPC) * np.float32(0.125)
    ck = np.tile(cos64, HPC)
    sq = np.tile(ssin64, HPC) * np.float32(0.125)
    sk = np.tile(ssin64, HPC)
    rope = np.tile(
        np.concatenate([cq, ck, sq, sk])[None, :], (B, 1)
    ).astype(np.float32)
    eall = np.zeros((B, B * 128), np.float32)
    for b in range(B):
        eall[b, 128 * b : 128 * (b + 1)] = 1.0
    c128 = np.ones((128, 2), np.float32)
    c128[:, 1] = 0.0
    c128[1, 1] = 1.0

    # weight layout: [partition=in-chunk-row, j=in-chunk, out-col],
    # contiguous per partition so DMA descriptors are large
    def wlay(w_rows):  # w_rows: (128, 1024) slice of W (rows = this core)
        return w_rows.T.reshape(8, 128, 128).transpose(1, 0, 2)

    xtl = np.ascontiguousarray(
        x.T.reshape(8, 128, B).transpose(1, 0, 2)
    ).astype(bf16)

    in_maps = []
    for c in range(NCORES):
        hs = slice(HPC * c, HPC * (c + 1))
        rs = slice(LP * c, LP * (c + 1))
        bqkv = np.tile(
            np.concatenate([bq[rs], bk[rs], bv[rs]])[None, :], (B, 1)
        ).astype(np.float32)
        c8 = np.concatenate([rope, bqkv], axis=1).astype(np.float32)
        ww = np.concatenate(
            [
                wlay(Wq[rs]),
                wlay(Wk[rs]),
                wlay(Wv[rs]),
                Wo[:, rs].reshape(8, 128, LP).transpose(2, 0, 1),
            ],
            axis=1,
        ).astype(bf16)
        # head-interleaved V: [b, partition, j, h, d]
        pvt = np.ascontiguousarray(
            past_v[:, hs]
            .reshape(B, HPC, 128, NCOL, HD)
            .transpose(0, 2, 3, 1, 4)
        ).astype(bf16)
        in_maps.append(
            {
                "xt": xtl,
                "ww": np.ascontiguousarray(ww),
                "c8": c8,
                "eb": eall.astype(bf16),
                "c128": c128,
                "pk": np.ascontiguousarray(past_k[:, hs]).astype(bf16),
                "pv": pvt,
            }
        )

    nc = _get_nc()
    trace = bool(int(os.environ.get("KERNEL_TRACE", "0")))
    if trace:
        _install_ntff_hook_shim()
    res = run_bass_kernel_spmd(
        nc, in_maps, core_ids=list(range(NCORES)), trace=trace
    )
    kernel.last_results = res

    partial = np.zeros((B, D), np.float32)
    for c in range(NCORES):
        partial = partial + res.results[c]["out"]
    out = partial + bo[None, :]
    return out.reshape(B, S, D).astype(np.float32)


# revision 26
# speedup vs baseline: 1.0965x; 1.0719x over previous
"""Bass/Trainium2 kernel for single-token (decode) self-attention with a
large KV cache, RoPE, and output projection.

Sharding: tensor-parallel over heads. 16 heads / 8 cores = 2 heads per
core; every core sees all 8 batch rows. Per-core HBM traffic is dominated
by its KV-cache slice, so the cache is down-converted on the host (pure
input marshaling): both K and V to bf16 (fp8 V was tried and costs
~1.8e-2 rel err - quantization noise on V hits the context at full
strength, it does not average away). QKV weights are sliced by head
rows, Wo by columns (row-parallel out projection); each core returns a
partial (8, 1024) output and the host sums the 8 partials.

Kernel structure per core:
  - q/k/v = x @ W.T + b via PE in bf16; all four weight slices arrive as
    one host-packed tensor so a single DMA covers them. RoPE on DVE in
    fp32 (q rows also carry the 1/sqrt(hd) attention scale), then the
    per-batch payload [q | v0 v1 | exp(s_new)] is downcast to bf16 and
    broadcast to all 128 partitions via one-hot PE matmuls.
  - K slabs land with key j = 64*partition + j_col, one DMA per
    (batch, head) on the SP HWDGE ring; V is host-interleaved by head
    ([p, j, h, d], 8KB descriptor chunks = exactly two 4KB packets) and
    streams on the ACT HWDGE ring so the two rings' issue gaps overlap.
    K is prefetched one batch ahead.
  - scores: plain tensor_tensor multiply against a 0-stride broadcast
    view of q, then a 6-step in-place binary-tree reduction over hd, also
    tensor_tensor. TT is the only two-tensor DVE op that reaches the 2x
    packed-bf16 mode on TRN2 hardware (measured: scalar_tensor_tensor
    always runs 1x; tensor_reduce has no fast mode at all).
  - softmax without max subtraction (scores are O(1) by construction);
    exp on ACT to bf16 weights in a head-interleaved [j, h] layout, with
    accum_out collecting each head's per-partition weight sum.
  - attn @ V: 64 block-diagonal PE matmuls per batch (stationary =
    at[:, j, 0:2] covering BOTH heads, moving = [v_h0 | v_h1], 128 cols;
    the off-diagonal PSUM blocks are garbage that is never read). This
    halves PE instruction count - the PE sequencer sustains only
    ~55ns/instruction and was the v5 bottleneck. Denominators accumulate
    into two extra PSUM columns via tiny f32 matmuls over the exp accums.
  - normalize on DVE two batches behind the score pipeline (a selector
    matmul first merges head 1's row down to partition 0), PE-transpose
    the context row, out-projection partial via bf16 PE.
"""

import functools
import os
import sys

import numpy as np

for _p in ("/opt/trn_rl_repo", "/root/.axon_site/_ro/trn_rl_repo"):
    if os.path.isdir(_p) and _p not in sys.path:
        sys.path.insert(0, _p)

from contextlib import ExitStack

import ml_dtypes

import concourse.tile as tile
from concourse import bacc, mybir
from concourse.bass_utils import run_bass_kernel_spmd

B, S, D, H, PAST = 8, 1, 1024, 16, 8192
HD = 64
NCORES = 8
HPC = H // NCORES          # heads per core = 2
LP = HPC * HD              # local projection width = 128
NCOL = PAST // 128         # 64 keys per partition = score columns per pair
PW = 2 * LP + HPC         # 258: [q(128) | v0 v1 (128) | exp(s_new)(2)]

F32 = mybir.dt.float32
BF16 = mybir.dt.bfloat16
MULT = mybir.AluOpType.mult
ADD = mybir.AluOpType.add
EXP = mybir.ActivationFunctionType.Exp


def _build_bass():
    nc = bacc.Bacc(
        "TRN2", target_bir_lowering=False, debug=False, num_devices=NCORES
    )

    # packed weights: [wq | wk | wv | wo] along dim1, 8 chunks each
    d_ww = nc.dram_tensor("ww", (128, 32, 128), BF16, kind="ExternalInput").ap()
    d_xt = nc.dram_tensor("xt", (128, 8, B), BF16, kind="ExternalInput").ap()
    # c8: [rope(512) | bqkv(384)] fp32 ; eallb: one-hot bcast rows, bf16
    d_c8 = nc.dram_tensor("c8", (B, 896), F32, kind="ExternalInput").ap()
    d_eb = nc.dram_tensor("eb", (B, B * 128), BF16, kind="ExternalInput").ap()
    # c128: col 0 = ones (transpose identity), col 1 = e1 row selector
    d_c128 = nc.dram_tensor("c128", (128, 2), F32, kind="ExternalInput").ap()
    d_pk = nc.dram_tensor("pk", (B, HPC, PAST, HD), BF16, kind="ExternalInput").ap()
    # V interleaved by head: [b, partition, j, h, d] so one matmul can
    # stream both heads' V rows for a key column (64-wide rows keep DMA
    # descriptors at exactly two 4KB packets)
    d_pv = nc.dram_tensor("pv", (B, 128, NCOL, HPC, HD), BF16, kind="ExternalInput").ap()
    d_out = nc.dram_tensor("out", (B, D), F32, kind="ExternalOutput").ap()

    with tile.TileContext(nc) as tc:
        with ExitStack() as ctx:
            const = ctx.enter_context(tc.tile_pool(name="const", bufs=1))
            small = ctx.enter_context(tc.tile_pool(name="small", bufs=1))
            tiny = ctx.enter_context(tc.tile_pool(name="tiny", bufs=2))
            wt = ctx.enter_context(tc.tile_pool(name="wt", bufs=1))
            kpool = ctx.enter_context(tc.tile_pool(name="kpool", bufs=4))
            vpool = ctx.enter_context(tc.tile_pool(name="vpool", bufs=3))
            prpool = ctx.enter_context(tc.tile_pool(name="prpool", bufs=2))
            atpool = ctx.enter_context(tc.tile_pool(name="atpool", bufs=2))

            # ---- constants + weights (ordered so the prologue chain can
            # start as early as possible) ----------------------------------
            wall = wt.tile([128, 32, 128], BF16)
            nc.sync.dma_start(wall[:], d_ww[:])
            xt = small.tile([128, 8, B], BF16)
            nc.scalar.dma_start(xt[:], d_xt[:])
            c8 = const.tile([B, 896], F32)
            nc.scalar.dma_start(c8[:], d_c8[:])
            eallb = const.tile([B, B * 128], BF16)
            nc.scalar.dma_start(eallb[:], d_eb[:])
            c128 = const.tile([128, 2], F32)
            nc.scalar.dma_start(c128[:], d_c128[:])
            ident = c128[:, 0:1]
            e1sel = c128[0:2, 1:2]
            rope = c8[:, 0:512]
            bias = c8[:, 512:896]
            wot = wall[:, 24:32, :]

            # prefetch K/V for batch 0 ahead of the prologue compute
            kts = {}
            vts = {}

            def fetch(b):
                kt = kpool.tile([128, HPC, NCOL, HD], BF16, tag="kt")
                ksrc = d_pk[b].rearrange("h (p j) d -> p h j d", p=128)
                nc.sync.dma_start(kt[:, 0], ksrc[:, 0])
                nc.sync.dma_start(kt[:, 1], ksrc[:, 1])
                kts[b] = kt
                vt = vpool.tile([128, NCOL, HPC, HD], BF16, tag="vt")
                nc.scalar.dma_start(
                    vt[:, 0 : NCOL // 2], d_pv[b, :, 0 : NCOL // 2]
                )
                nc.scalar.dma_start(
                    vt[:, NCOL // 2 :], d_pv[b, :, NCOL // 2 :]
                )
                vts[b] = vt

            fetch(0)

            # ---- prologue: projections, RoPE, payload broadcast -----------
            qb = const.tile([128, B * PW], BF16)
            with ExitStack() as pctx:
                ps_p = pctx.enter_context(
                    tc.tile_pool(name="ps_p", bufs=1, space="PSUM")
                )
                ps_bc = pctx.enter_context(
                    tc.tile_pool(name="ps_bc", bufs=2, space="PSUM")
                )

                # qkv projection: (8, 384) = x @ [Wq|Wk|Wv].T
                qkv_ps = ps_p.tile([B, 3 * LP], F32, tag="qkv_ps")
                for i in range(3):
                    for j in range(8):
                        nc.tensor.matmul(
                            qkv_ps[:, LP * i : LP * (i + 1)],
                            xt[:, j, :],
                            wall[:, 8 * i + j, :],
                            start=(j == 0),
                            stop=(j == 7),
                        )
                qkv = small.tile([B, 3 * LP], F32)
                nc.vector.tensor_tensor(qkv[:], qkv_ps[:], bias[:], ADD)

                # RoPE on q and k (fp32): rot = qk * C + swapped(qk) * S
                rot = small.tile([B, 2 * LP], F32)
                swp = small.tile([B, 2 * LP], F32)
                for i in range(2):  # q, k
                    src = qkv[:, LP * i : LP * (i + 1)].rearrange(
                        "p (h t f) -> p h t f", h=HPC, t=2
                    )
                    dst = swp[:, LP * i : LP * (i + 1)].rearrange(
                        "p (h t f) -> p h t f", h=HPC, t=2
                    )
                    nc.vector.tensor_copy(dst[:, :, 0, :], src[:, :, 1, :])
                    nc.vector.tensor_copy(dst[:, :, 1, :], src[:, :, 0, :])
                tmp = small.tile([B, 2 * LP], F32)
                nc.vector.tensor_tensor(tmp[:], swp[:], rope[:, 256:512], MULT)
                nc.vector.tensor_tensor(
                    rot[:], qkv[:, 0 : 2 * LP], rope[:, 0:256], MULT
                )
                nc.vector.tensor_tensor(rot[:], rot[:], tmp[:], ADD)

                # new-token scores s_new = rot(q) . rot(k) per head
                # (q side is pre-scaled by 0.125 via the rope tables)
                snew = small.tile([B, HPC], F32)
                sttp = small.tile([B, HD], F32)
                for hp in range(HPC):
                    nc.vector.scalar_tensor_tensor(
                        out=sttp[:],
                        in0=rot[:, LP + HD * hp : LP + HD * (hp + 1)],
                        scalar=1.0,
                        in1=rot[:, HD * hp : HD * (hp + 1)],
                        op0=MULT,
                        op1=MULT,
                        accum_out=snew[:, hp : hp + 1],
                    )

                # payload (bf16): [q(128) | v0 1 | v1 1 | exp(s_new)(2)]
                payb = small.tile([B, PW], BF16)
                nc.vector.tensor_copy(payb[:, 0:LP], rot[:, 0:LP])
                nc.vector.tensor_copy(
                    payb[:, LP : 2 * LP], qkv[:, 2 * LP : 3 * LP]
                )
                nc.scalar.activation(
                    payb[:, PW - HPC : PW], snew[:], EXP
                )

                # broadcast payload rows to all 128 partitions
                for b in range(B):
                    bc = ps_bc.tile([128, PW], F32, tag="bc")
                    nc.tensor.matmul(
                        bc[:],
                        eallb[:, 128 * b : 128 * (b + 1)],
                        payb[:],
                        start=True,
                        stop=True,
                    )
                    nc.scalar.copy(qb[:, PW * b : PW * (b + 1)], bc[:])

            # ---- main loop: one iteration per batch row (2 heads each) ----
            ps_ctx = ctx.enter_context(
                tc.tile_pool(name="ps_ctx", bufs=3, space="PSUM")
            )
            ps_m = ctx.enter_context(
                tc.tile_pool(name="ps_m", bufs=2, space="PSUM")
            )
            ps_t = ctx.enter_context(
                tc.tile_pool(name="ps_t", bufs=1, space="PSUM")
            )
            ps_o = ctx.enter_context(
                tc.tile_pool(name="ps_o", bufs=2, space="PSUM")
            )
            ctxT_ps = ps_t.tile([128, B], F32)
            ctxn = small.tile([1, B * LP], F32)
            ctxbs = {}

            def epilogue(b):
                # ctxb is [2, 128]: head 0 context on row 0 cols 0:64, head 1
                # on row 1 cols 64:128 (block-diagonal accumulation; the
                # off-diagonal blocks are never read). Merge row 1 down to
                # partition 0 via a selector matmul, then normalize by the
                # denominators collected in den.
                ctxb = ctxbs.pop(b)
                sb = tiny.tile([HPC, HPC * HD + HPC], F32, tag="sb")
                nc.scalar.copy(sb[:], ctxb[:])
                m1 = ps_m.tile([1, HPC * HD + HPC], F32, tag="m1")
                nc.tensor.matmul(m1[:], e1sel, sb[:], start=True, stop=True)
                rec = tiny.tile([1, HPC], F32, tag="rec")
                nc.vector.reciprocal(rec[:], ctxb[0:1, HPC * HD : HPC * HD + HPC])
                nc.vector.tensor_scalar_mul(
                    ctxn[0:1, LP * b : LP * b + HD],
                    sb[0:1, 0:HD],
                    rec[0:1, 0:1],
                )
                nc.vector.tensor_scalar_mul(
                    ctxn[0:1, LP * b + HD : LP * (b + 1)],
                    m1[0:1, HD : 2 * HD],
                    rec[0:1, 1:2],
                )
                nc.tensor.transpose(
                    ctxT_ps[:, b : b + 1],
                    ctxn[0:1, LP * b : LP * (b + 1)],
                    ident[0:1, 0:1],
                )

            for b in range(B):
                q0 = PW * b
                kt = kts.pop(b)
                vt = vts.pop(b)
                if b + 1 < B:
                    fetch(b + 1)

                # scores: prod = kt * q (2x DVE) per head, then one fused
                # in-place tree-reduce over d covering both heads
                prod = prpool.tile([128, HPC, NCOL, HD], BF16, tag="prod")
                at = atpool.tile([128, NCOL, HPC], BF16, tag="at")
                acc = atpool.tile([128, HPC], F32, tag="acc")
                for h in range(HPC):
                    qv = qb[:, q0 + HD * h : q0 + HD * (h + 1)].rearrange(
                        "p (o d) -> p o d", o=1
                    ).broadcast_to([128, NCOL, HD])
                    nc.vector.tensor_tensor(prod[:, h], kt[:, h], qv, MULT)
                # normalize batch b-2 now: early in the queues so its
                # ACT copy is not stuck behind this iteration's exp and its
                # DVE ops slot in right after the tree
                if b >= 2:
                    epilogue(b - 2)
                pf = prod.rearrange("p h j d -> p (h j) d")
                w = HD // 2
                while w >= 1:
                    nc.vector.tensor_tensor(
                        pf[:, :, 0:w], pf[:, :, 0:w], pf[:, :, w : 2 * w], ADD
                    )
                    w //= 2
                # exp into the head-interleaved at layout; accum_out gives
                # each head's per-partition weight sum for the denominator
                for h in range(HPC):
                    nc.scalar.activation(
                        at[:, :, h],
                        prod[:, h, :, 0],
                        EXP,
                        accum_out=acc[:, h : h + 1],
                    )

                # attn @ V on PE, both heads per matmul (block-diagonal):
                # stationary at[:, j, 0:2], moving [v_h0 | v_h1] (128 cols)
                # cols 0:128 = block-diagonal context, 128:130 = denoms
                ctxb = ps_ctx.tile([HPC, HPC * HD + HPC], F32, tag="ctxb")
                ctxbs[b] = ctxb
                for j in range(NCOL):
                    nc.tensor.matmul(
                        ctxb[:, 0 : HPC * HD],
                        at[:, j, :],
                        vt[:, j],
                        start=(j == 0),
                        stop=False,
                    )
                # new token, both heads at once: lhsT = [es0 es1],
                # rhs = [v0 | v1]
                nc.tensor.matmul(
                    ctxb[:, 0 : HPC * HD],
                    qb[0:1, q0 + PW - HPC : q0 + PW],
                    qb[0:1, q0 + LP : q0 + 2 * LP],
                    start=False,
                    stop=True,
                )
                # denominators: cross-partition sum of acc plus exp(s_new)
                for h in range(HPC):
                    dsl = ctxb[0:1, HPC * HD + h : HPC * HD + h + 1]
                    nc.tensor.matmul(
                        dsl, acc[:, h : h + 1], ident, start=True, stop=False
                    )
                    nc.tensor.matmul(
                        dsl,
                        eallb[0:1, 0:1],
                        qb[0:1, q0 + PW - HPC + h : q0 + PW - HPC + h + 1],
                        start=False,
                        stop=True,
                    )

            epilogue(B - 2)
            epilogue(B - 1)

            # ---- finalize: transpose is done; out-projection ---------------
            ctxT = small.tile([128, B], BF16)
            nc.scalar.copy(ctxT[:], ctxT_ps[:])

            outsb = small.tile([B, D], F32)
            for half in range(2):
                op_ps = ps_o.tile([B, 512], F32, tag="op_ps")
                nc.tensor.matmul(
                    op_ps[:],
                    ctxT[:],
                    wot[:, 4 * half : 4 * (half + 1), :],
                    start=True,
                    stop=True,
                )
                nc.vector.tensor_copy(
                    outsb[:, 512 * half : 512 * (half + 1)], op_ps[:]
                )
                nc.sync.dma_start(
                    d_out[:, 512 * half : 512 * (half + 1)],
                    outsb[:, 512 * half : 512 * (half + 1)],
                )

    nc.compile()
    return nc


@functools.lru_cache(maxsize=1)
def _get_nc():
    return _build_bass()


def _rope_tables():
    """cos/sin rows for position PAST, mirroring reference.py's fp32 jax
    arithmetic so the tables round identically."""
    import jax
    import jax.numpy as jnp

    pos = (PAST + jnp.arange(S)).astype(jnp.float32)
    inv_freq = 1.0 / (
        10000.0 ** (jnp.arange(0, HD, 2, dtype=jnp.float32) / HD)
    )
    ang = pos[:, None] * inv_freq[None, :]
    cos32 = np.asarray(jnp.cos(ang))[0]
    sin32 = np.asarray(jnp.sin(ang))[0]
    cos64 = np.concatenate([cos32, cos32])
    ssin64 = np.concatenate([-sin32, sin32])
    return cos64.astype(np.float32), ssin64.astype(np.float32)


def _install_ntff_hook_shim():
    """The agent image's antenv stub lacks axon_hooks, which degrades
    run_bass_kernel_spmd(trace=True) into an ImportError. Provide the
    module and register the ctypes-based NTFF hook from trn_agent_boot."""
    import types

    try:
        import antenv.axon_hooks  # noqa: F401

        return
    except ImportError:
        pass
    try:
        import antenv
        from trn_agent_boot.trn_boot import _ntff_profile_via_ctypes

        mod = types.ModuleType("antenv.axon_hooks")
        _state = {"hook": _ntff_profile_via_ctypes("/opt/axon/libaxon_pjrt.so")}
        mod.get_axon_ntff_profile_hook = lambda: _state["hook"]
        mod.set_axon_ntff_profile_hook = lambda h: _state.update(hook=h)
        sys.modules["antenv.axon_hooks"] = mod
        antenv.axon_hooks = mod
    except Exception as e:  # profiling is best-effort
        print(f"ntff hook shim failed: {e}", file=sys.stderr)


def kernel(x, Wq, bq, Wk, bk, Wv, bv, Wo, bo, past_k, past_v):
    x = np.asarray(x, np.float32).reshape(B, D)
    Wq = np.asarray(Wq, np.float32)
    Wk = np.asarray(Wk, np.float32)
    Wv = np.asarray(Wv, np.float32)
    Wo = np.asarray(Wo, np.float32)
    bq = np.asarray(bq, np.float32)
    bk = np.asarray(bk, np.float32)
    bv = np.asarray(bv, np.float32)
    bo = np.asarray(bo, np.float32)
    past_k = np.asarray(past_k, np.float32)
    past_v = np.asarray(past_v, np.float32)

    bf16 = ml_dtypes.bfloat16

    cos64, ssin64 = _rope_tables()
    # C/S for the q columns carry the 1/sqrt(hd) attention scale
    cq = np.tile(cos64, HPC) * np.float32(0.125)
    ck = np.tile(cos64, HPC)
    sq = np.tile(ssin64, HPC) * np.float32(0.125)
    sk = np.tile(ssin64, HPC)
    rope = np.tile(
        np.concatenate([cq, ck, sq, sk])[None, :], (B, 1)
    ).astype(np.float32)
    eall = np.zeros((B, B * 128), np.float32)
    for b in range(B):
        eall[b, 128 * b : 128 * (b + 1)] = 1.0
    c128 = np.ones((128, 2), np.float32)
    c128[:, 1] = 0.0
    c128[1, 1] = 1.0

    # weight layout: [partition=in-chunk-row, j=in-chunk, out-col],
    # contiguous per partition so DMA descriptors are large
    def wlay(w_rows):  # w_rows: (128, 1024) slice of W (rows = this core)
        return w_rows.T.reshape(8, 128, 128).transpose(1, 0, 2)

    xtl = np.ascontiguousarray(
        x.T.reshape(8, 128, B).transpose(1, 0, 2)
    ).astype(bf16)

    in_maps = []
    for c in range(NCORES):
        hs = slice(HPC * c, HPC * (c + 1))
        rs = slice(LP * c, LP * (c + 1))
        bqkv = np.tile(
            np.concatenate([bq[rs], bk[rs], bv[rs]])[None, :], (B, 1)
        ).astype(np.float32)
        c8 = np.concatenate([rope, bqkv], axis=1).astype(np.float32)
        ww = np.concatenate(
            [
                wlay(Wq[rs]),
                wlay(Wk[rs]),
                wlay(Wv[rs]),
                Wo[:, rs].reshape(8, 128, LP).transpose(2, 0, 1),
            ],
            axis=1,
        ).astype(bf16)
        # head-interleaved V: [b, partition, j, h, d]
        pvt = np.ascontiguousarray(
            past_v[:, hs]
            .reshape(B, HPC, 128, NCOL, HD)
            .transpose(0, 2, 3, 1, 4)
        ).astype(bf16)
        in_maps.append(
            {
                "xt": xtl,
                "ww": np.ascontiguousarray(ww),
                "c8": c8,
                "eb": eall.astype(bf16),
                "c128": c128,
                "pk": np.ascontiguousarray(past_k[:, hs]).astype(bf16),
                "pv": pvt,
            }
        )

    nc = _get_nc()
    trace = bool(int(os.environ.get("KERNEL_TRACE", "0")))
    if trace:
        _install_ntff_hook_shim()
    res = run_bass_kernel_spmd(
        nc, in_maps, core_ids=list(range(NCORES)), trace=trace
    )
    kernel.last_results = res

    partial = np.zeros((B, D), np.float32)
    for c in range(NCORES):
        partial = partial + res.results[c]["out"]
    out = partial + bo[None, :]
    return out.reshape(B, S, D).astype(np.float32)
